# revision 1
# baseline (speedup 1.0000x reference)
"""HadLinear Trainium2 kernel: per-128-block L2-norm -> Hadamard -> 4-bit
Gaussian-codebook quantize -> rescale -> matmul with w.T/sqrt(128).

Sharding: 8-way data-parallel over tokens (16384 tokens / 8 cores = 2048 each).
Weight is host-pre-transposed to [in_dim, out_dim] fp16 and replicated.
All quantization arithmetic (norms, Hadamard, bucket compares) is fp32 on
device; only the final matmul operands are fp16.
"""

import math

import numpy as np

# ---------------------------------------------------------------- constants
BS = 128          # hadamard block size
NLEVELS = 16

_consts = None


def _get_consts():
    global _consts
    if _consts is not None:
        return _consts
    import jax

    _p = (np.arange(NLEVELS) + 0.5) / NLEVELS
    # mirror reference.py exactly (fp32 jax ppf)
    cent = np.asarray(jax.scipy.stats.norm.ppf(_p), dtype=np.float32)
    bound = np.asarray(
        (np.asarray(cent[1:]) + np.asarray(cent[:-1])) * np.float32(0.5),
        dtype=np.float32,
    )
    # positive half: cpos = cent[8:16]; positive boundaries bound[8:15]
    cpos = cent[8:16].copy()
    bpos = bound[8:15].copy()
    dpos = (cpos[1:] - cpos[:-1]).astype(np.float32)  # 7 deltas
    _consts = (cent, bound, cpos, bpos, dpos)
    return _consts


def _hadamard_matrix():
    x = np.eye(BS, dtype=np.float32)
    h = 1
    while h < BS:
        x = x.reshape(BS, -1, 2, h)
        a, b = x[:, :, 0, :], x[:, :, 1, :]
        x = np.concatenate([a + b, a - b], axis=-1)
        h *= 2
    return np.ascontiguousarray(x.reshape(BS, BS))  # out_row = e_i -> M[i, :]


# ---------------------------------------------------------------- builder
def build_module(tok, d, gt, nchunk_n=512, num_devices=8, n_act_masks=5):
    """Build the per-core bass program.

    tok: tokens per core; d: feature dim; gt: tokens per matmul group.
    """
    import concourse.bass as bass
    import concourse.tile as tile
    from concourse import bacc, mybir

    f32 = mybir.dt.float32
    f16 = mybir.dt.float16
    A = mybir.AluOpType

    nb = d // BS                    # 128-blocks per row
    ntile = tok // 128              # token tiles
    ngroup = tok // gt              # matmul groups
    tpg = gt // 128                 # token tiles per group
    nn = d // nchunk_n              # output-col chunks
    ncht = d // 512                 # 512-col psum chunks per token tile
    qc = min(1024, d)               # staircase quarter cols
    nq = d // qc
    cpq = qc // 512                 # psum chunks per quarter
    bpq = qc // 128                 # blocks per quarter
    hcols = min(2048, d)            # x-load half cols
    nh = d // hcols

    _, _, cpos, bpos, dpos = _get_consts()
    n_dve = 7 - n_act_masks
    dve_idx = list(range(n_dve))
    act_idx = list(range(n_dve, 7))
    const0 = np.float32(cpos[0])
    for i in act_idx:
        const0 = np.float32(const0 + np.float32(dpos[i] * np.float32(0.5)))

    nc = bacc.Bacc(
        "TRN2", target_bir_lowering=False, debug=False,
        num_devices=num_devices,
    )
    x_in = nc.dram_tensor("x_in", [tok, d], f32, kind="ExternalInput").ap()
    w_t = nc.dram_tensor("w_t", [d, d], f16, kind="ExternalInput").ap()
    hmat_d = nc.dram_tensor("hmat", [BS, BS], f32, kind="ExternalInput").ap()
    iden_d = nc.dram_tensor("iden", [BS, BS], f32, kind="ExternalInput").ap()
    out = nc.dram_tensor("out", [tok, d], f32, kind="ExternalOutput").ap()

    wt_v = w_t.rearrange("(k p) n -> p k n", p=BS)  # [128, nb, d]

    with tile.TileContext(nc) as tc:
        import contextlib

        ctx = contextlib.ExitStack()
        with ctx:
            singles = ctx.enter_context(tc.tile_pool(name="singles", bufs=1))
            xin_p = ctx.enter_context(tc.tile_pool(name="xin", bufs=2))
            sqs_p = ctx.enter_context(tc.tile_pool(name="sqs", bufs=2))
            sm_p = ctx.enter_context(tc.tile_pool(name="sm", bufs=2))
            xn_p = ctx.enter_context(tc.tile_pool(name="xn", bufs=4))
            xnt_p = ctx.enter_context(tc.tile_pool(name="xnt", bufs=2))
            axh_p = ctx.enter_context(tc.tile_pool(name="axh", bufs=2))
            sgn_p = ctx.enter_context(tc.tile_pool(name="sgn", bufs=2))
            acc_p = ctx.enter_context(tc.tile_pool(name="acc", bufs=2))
            msk_p = ctx.enter_context(tc.tile_pool(name="msk", bufs=2))
            s2b_p = ctx.enter_context(tc.tile_pool(name="s2b", bufs=2))
            s2t_p = ctx.enter_context(tc.tile_pool(name="s2t", bufs=2))
            xqg_p = ctx.enter_context(tc.tile_pool(name="xqg", bufs=2))
            w_p = ctx.enter_context(tc.tile_pool(name="wsl", bufs=2))
            ev_p = ctx.enter_context(tc.tile_pool(name="ev", bufs=3))
            tp_p = ctx.enter_context(
                tc.tile_pool(name="tp", bufs=2, space="PSUM"))
            hp_p = ctx.enter_context(
                tc.tile_pool(name="hp", bufs=2, space="PSUM"))
            sp_p = ctx.enter_context(
                tc.tile_pool(name="sp", bufs=1, space="PSUM"))
            mp_p = ctx.enter_context(
                tc.tile_pool(name="mp", bufs=2, space="PSUM"))
            dr_p = ctx.enter_context(
                tc.tile_pool(name="dr", bufs=4, space="DRAM"))

            hmat_s = singles.tile([BS, BS], f32)
            iden_s = singles.tile([BS, BS], f32)
            nc.sync.dma_start(out=hmat_s[:], in_=hmat_d[:, :])
            nc.sync.dma_start(out=iden_s[:], in_=iden_d[:, :])
            nbias = {}
            for i in act_idx:
                bt = singles.tile([BS, 1], f32, tag=f"nb{i}")
                nc.vector.memset(bt[:], float(-bpos[i]))
                nbias[i] = bt

            def quantize_group(g):
                xqg = xqg_p.tile([BS, nb, gt], f16, tag="xqg")
                for tt in range(tpg):
                    trow = g * gt + tt * 128
                    xh_tiles = []
                    normsq = sm_p.tile([BS, nb], f32, tag="normsq")
                    for h in range(nh):
                        xh_t = xin_p.tile([BS, hcols], f32, tag="xin")
                        nc.sync.dma_start(
                            out=xh_t[:],
                            in_=x_in[trow:trow + 128,
                                     h * hcols:(h + 1) * hcols])
                        xh_tiles.append(xh_t)
                        for bb in range(hcols // 128):
                            b = h * (hcols // 128) + bb
                            scr = sqs_p.tile([BS, 128], f32, tag="sqs")
                            nc.scalar.activation(
                                out=scr[:],
                                in_=xh_t[:, bb * 128:(bb + 1) * 128],
                                func=mybir.ActivationFunctionType.Square,
                                accum_out=normsq[:, b:b + 1])
                    # s = sqrt(normsq) with one Newton step; inv_s; s2=s/128
                    y0 = sm_p.tile([BS, nb], f32, tag="y0")
                    nc.scalar.activation(
                        out=y0[:], in_=normsq[:],
                        func=mybir.ActivationFunctionType.Sqrt)
                    r = sm_p.tile([BS, nb], f32, tag="r")
                    nc.vector.reciprocal(out=r[:], in_=y0[:])
                    t = sm_p.tile([BS, nb], f32, tag="t")
                    nc.vector.tensor_mul(t[:], normsq[:], r[:])
                    u = sm_p.tile([BS, nb], f32, tag="u")
                    nc.vector.tensor_add(u[:], y0[:], t[:])
                    y1 = sm_p.tile([BS, nb], f32, tag="y1")
                    nc.vector.tensor_scalar_mul(y1[:], u[:], 0.5)
                    inv_s = sm_p.tile([BS, nb], f32, tag="invs")
                    nc.vector.reciprocal(out=inv_s[:], in_=y1[:])
                    s2 = sm_p.tile([BS, nb], f32, tag="s2")
                    nc.vector.tensor_scalar_mul(y1[:] if False else s2[:],
                                                y1[:], 1.0 / 128.0)
                    # s2 transposed to [nb, 128] then to DRAM for bcast loads
                    sps = sp_p.tile([nb, BS], f32, tag="sp")
                    nc.tensor.transpose(sps[:], s2[:], iden_s[:])
                    s2t = s2t_p.tile([nb, BS], f32, tag="s2t")
                    nc.scalar.copy(out=s2t[:], in_=sps[:])
                    s2d = dr_p.tile([nb, BS], f32, tag="s2d")
                    nc.sync.dma_start(out=s2d[:], in_=s2t[:])

                    for q in range(nq):
                        axh = axh_p.tile([BS, qc], f32, tag="axh")
                        sgn = sgn_p.tile([BS, qc], f32, tag="sgn")
                        for cc in range(cpq):
                            c = q * cpq + cc
                            tp = tp_p.tile([BS, 512], f32, tag="tp")
                            for j in range(4):
                                b = c * 4 + j
                                h = b // (hcols // 128)
                                bb = b % (hcols // 128)
                                xnb = xn_p.tile([BS, 128], f32, tag="xn")
                                nc.vector.tensor_scalar_mul(
                                    xnb[:],
                                    xh_tiles[h][:, bb * 128:(bb + 1) * 128],
                                    inv_s[:, b:b + 1])
                                nc.tensor.transpose(
                                    tp[:, j * 128:(j + 1) * 128], xnb[:],
                                    iden_s[:])
                            xnt = xnt_p.tile([BS, 512], f32, tag="xnt")
                            nc.scalar.copy(out=xnt[:], in_=tp[:])
                            hp = hp_p.tile([BS, 512], f32, tag="hp")
                            nc.tensor.matmul(hp[:], lhsT=hmat_s[:],
                                             rhs=xnt[:], start=True,
                                             stop=True)
                            nc.scalar.activation(
                                out=axh[:, cc * 512:(cc + 1) * 512],
                                in_=hp[:],
                                func=mybir.ActivationFunctionType.Abs)
                            nc.scalar.activation(
                                out=sgn[:, cc * 512:(cc + 1) * 512],
                                in_=hp[:],
                                func=mybir.ActivationFunctionType.Sign)
                        # ---- staircase (exact fp32 compares) ----
                        acc = acc_p.tile([BS, qc], f32, tag="acc")
                        nc.vector.tensor_scalar(
                            out=acc[:], in0=axh[:],
                            scalar1=float(bpos[dve_idx[0]]),
                            scalar2=float(dpos[dve_idx[0]]),
                            op0=A.is_gt, op1=A.mult)
                        for i in dve_idx[1:]:
                            mk = msk_p.tile([BS, qc], f32, tag="msk")
                            nc.vector.tensor_scalar(
                                out=mk[:], in0=axh[:],
                                scalar1=float(bpos[i]),
                                scalar2=float(dpos[i]),
                                op0=A.is_gt, op1=A.mult)
                            nc.vector.tensor_add(acc[:], acc[:], mk[:])
                        for i in act_idx:
                            mk = msk_p.tile([BS, qc], f32, tag="msk")
                            nc.scalar.activation(
                                out=mk[:], in_=axh[:],
                                func=mybir.ActivationFunctionType.Sign,
                                bias=nbias[i][:], scale=1.0)
                            nc.vector.scalar_tensor_tensor(
                                out=acc[:], in0=mk[:],
                                scalar=float(dpos[i] * np.float32(0.5)),
                                in1=acc[:], op0=A.mult, op1=A.add)
                        # sigma = sgn * s2_broadcast ; xq = (acc+c0)*sigma
                        s2b = s2b_p.tile([BS, qc], f32, tag="s2b")
                        src = s2d[:]
                        bcast = bass.AP(
                            tensor=src.tensor,
                            offset=src.offset + q * bpq * BS,
                            ap=[[0, BS], [BS, bpq], [1, BS]])
                        nc.sync.dma_start(out=s2b[:], in_=bcast)
                        nc.vector.tensor_mul(sgn[:], sgn[:], s2b[:])
                        xq_v = xqg[:, q * bpq:(q + 1) * bpq,
                                   tt * 128:(tt + 1) * 128]
                        acc3 = acc[:].rearrange("p (b t) -> p b t", t=128)
                        sgn3 = sgn[:].rearrange("p (b t) -> p b t", t=128)
                        nc.vector.scalar_tensor_tensor(
                            out=xq_v, in0=acc3, scalar=float(const0),
                            in1=sgn3, op0=A.add, op1=A.mult)
                return xqg

            def matmul_group(g, xqg):
                for n in range(nn):
                    wsl = w_p.tile([BS, nb, nchunk_n], f16, tag="wsl")
                    for k in range(nb):
                        nc.sync.dma_start(
                            out=wsl[:, k, :],
                            in_=wt_v[:, k, n * nchunk_n:(n + 1) * nchunk_n])
                    for m in range(tpg):
                        ps = mp_p.tile([BS, nchunk_n], f32, tag="mp")
                        for k in range(nb):
                            nc.tensor.matmul(
                                ps[:],
                                lhsT=xqg[:, k, m * 128:(m + 1) * 128],
                                rhs=wsl[:, k, :],
                                start=(k == 0), stop=(k == nb - 1))
                        ev = ev_p.tile([BS, nchunk_n], f32, tag="ev")
                        nc.scalar.copy(out=ev[:], in_=ps[:])
                        nc.sync.dma_start(
                            out=out[g * gt + m * 128:g * gt + (m + 1) * 128,
                                    n * nchunk_n:(n + 1) * nchunk_n],
                            in_=ev[:])

            # emission order: Q0 Q1 M0 Q2 M1 Q3 M2 M3  (PE overlap)
            pend = []
            pend.append(quantize_group(0))
            for g in range(1, ngroup):
                pend.append(quantize_group(g))
                matmul_group(g - 1, pend[g - 1])
            matmul_group(ngroup - 1, pend[ngroup - 1])

    nc.compile()
    return nc


# ---------------------------------------------------------------- driver
_CACHED = None

TOK_FULL = 2048
D_FULL = 4096
GT_FULL = 512


def _get_compiled():
    global _CACHED
    if _CACHED is None:
        from concourse.bass_interp import get_hw_module

        nc = build_module(TOK_FULL, D_FULL, GT_FULL, num_devices=8)
        nc.m = get_hw_module(nc.m)
        _CACHED = nc
    return _CACHED


def _run(input, weight, trace=False):
    from concourse import bass_utils

    nc = _get_compiled()
    x = np.ascontiguousarray(
        np.asarray(input, dtype=np.float32).reshape(-1, D_FULL))
    wt = np.ascontiguousarray(
        np.asarray(weight, dtype=np.float32).T).astype(np.float16)
    hm = _hadamard_matrix()
    iden = np.ascontiguousarray(np.eye(BS, dtype=np.float32))
    ncores = 8
    in_maps = [
        {"x_in": np.ascontiguousarray(x[i * TOK_FULL:(i + 1) * TOK_FULL]),
         "w_t": wt, "hmat": hm, "iden": iden}
        for i in range(ncores)
    ]
    res = bass_utils.run_bass_kernel_spmd(
        nc, in_maps, core_ids=list(range(ncores)), trace=trace)
    outs = [res.results[i]["out"] for i in range(ncores)]
    full = np.concatenate(outs, axis=0).reshape(input.shape)
    return full, res


def kernel(input, weight):
    out, _ = _run(input, weight, trace=False)
    return out

